# revision 1
# baseline (speedup 1.0000x reference)
"""A3TGCN kernel for 8 TRN2 NeuronCores (Bass/Tile).

Self-contained: host-side layout prep (norm folding, sorting/padding/packing
of the edge list into a per-block chunk grid), an SPMD Bass/Tile kernel
(node-partitioned graph aggregation via hardware dma_gather + batched one-hot
PE-matmul scatter, plus a transposed GRU recurrence), and unsharding.

kernel(**inputs) -> np.ndarray [B, N, F_OUT] float32
"""
import contextlib
import numpy as np
import ml_dtypes

bf16 = ml_dtypes.bfloat16


class Cfg:
    def __init__(self, N=50000, E=800000, B=2, F_IN=2, F_OUT=32, T=12,
                 NCORES=8, MACRO=1024):
        self.N, self.E, self.B, self.F_IN, self.F_OUT, self.T = N, E, B, F_IN, F_OUT, T
        self.NCORES = NCORES
        assert N % NCORES == 0
        self.CPN = N // NCORES
        self.BLOCKS = (self.CPN + 127) // 128
        self.CPNP = self.BLOCKS * 128
        self.NP_TOT = NCORES * self.CPNP
        self.NPAIR = self.NP_TOT // 2
        assert self.NP_TOT % 2 == 0 and self.NPAIR <= 32767
        self.FW = B * F_IN * T          # 48
        self.MACRO = MACRO
        assert MACRO % 128 == 0

    def gp_of(self, n):
        return (n // self.CPN) * self.CPNP + (n % self.CPN)


FULL = Cfg()


def compute_dinv(cfg, edge_index, edge_weight):
    """deg (incl self loop) and its inverse sqrt, host-side."""
    col = np.asarray(edge_index[1], np.int64)
    w = np.asarray(edge_weight, np.float64)
    deg = np.bincount(col, weights=w, minlength=cfg.N) + 1.0
    return (1.0 / np.sqrt(deg)).astype(np.float32)


def core_edges(cfg, edge_index, edge_weight, core):
    """Edges (incl self loops) owned by `core`, sorted by local col."""
    N = cfg.N
    row = np.asarray(edge_index[0], np.int64)
    col = np.asarray(edge_index[1], np.int64)
    w = np.asarray(edge_weight, np.float32)
    loop = np.arange(N, dtype=np.int64)
    row = np.concatenate([row, loop])
    col = np.concatenate([col, loop])
    w = np.concatenate([w, np.ones(N, np.float32)])
    own = (col // cfg.CPN) == core
    row, col, w = row[own], col[own], w[own]
    c_loc = (col % cfg.CPN).astype(np.int64)
    order = np.argsort(c_loc, kind="stable")
    return row[order], w[order], c_loc[order]


def compute_grid(cfg, edge_index, edge_weight):
    """Per-block chunk counts: max over cores (shared compiled program), plus
    macro-alignment padding appended to the last block."""
    cnt = np.zeros((cfg.NCORES, cfg.BLOCKS), np.int64)
    for core in range(cfg.NCORES):
        _, _, c_loc = core_edges(cfg, edge_index, edge_weight, core)
        cnt[core] = np.bincount(c_loc // 128, minlength=cfg.BLOCKS)
    kblk = ((cnt.max(axis=0) + 127) // 128).astype(np.int64)
    kblk = np.maximum(kblk, 1)
    C = int(kblk.sum())
    CPMAC = cfg.MACRO // 128
    pad = (-C) % CPMAC
    kblk[-1] += pad
    C += pad
    chunk_blk = np.repeat(np.arange(cfg.BLOCKS), kblk)
    return kblk, C, chunk_blk


def prep_core(cfg, edge_index, edge_weight, dinv, core, kblk):
    """Per-core slot stream on the shared grid; norm folded into weights."""
    row, w, c_loc = core_edges(cfg, edge_index, edge_weight, core)
    col_g = core * cfg.CPN + c_loc
    wn = (w * dinv[col_g]).astype(np.float32)

    blk = c_loc // 128
    rows_out, w_out, cloc_out = [], [], []
    for bI in range(cfg.BLOCKS):
        m = blk == bI
        r_b, w_b, c_b = row[m], wn[m], c_loc[m]
        pad = int(kblk[bI]) * 128 - len(r_b)
        assert pad >= 0
        if pad:
            r_b = np.concatenate([r_b, np.full(pad, cfg.N - 1, np.int64)])
            w_b = np.concatenate([w_b, np.zeros(pad, np.float32)])
            c_b = np.concatenate([c_b, np.full(pad, bI * 128, np.int64)])
        rows_out.append(r_b); w_out.append(w_b); cloc_out.append(c_b)
    row = np.concatenate(rows_out)
    wn = np.concatenate(w_out)
    c_loc = np.concatenate(cloc_out)
    S = len(row)
    C = S // 128

    gp = cfg.gp_of(row)
    pair = (gp >> 1).astype(np.int16)
    parity = (gp & 1).astype(np.uint8)

    def to_pc(a, dt):
        return np.ascontiguousarray(a.reshape(C, 128).T.astype(dt))

    w_even = np.where(parity == 0, wn, 0).astype(np.float32)
    w_odd = np.where(parity == 1, wn, 0).astype(np.float32)
    # gather indices wrapped in 16 partitions (replicated on device)
    gidx16 = np.ascontiguousarray(pair.reshape(S // 16, 16).T)
    return dict(
        w_even=to_pc(w_even, bf16),
        w_odd=to_pc(w_odd, bf16),
        colv=to_pc((c_loc % 128).astype(np.float32), bf16),
        gidx16=gidx16,
        S=S, C=C,
    )


def prep_x(cfg, X, dinv):
    """X [B,N,F,T] f32 -> Xs [NP_TOT, 48] bf16, row gp, col j = t*4+b*2+f,
    pre-scaled by dinv[src]."""
    B, N, F_IN, T = cfg.B, cfg.N, cfg.F_IN, cfg.T
    Xf = np.transpose(np.asarray(X, np.float32), (3, 0, 2, 1))  # [T,B,F,N]
    Xf = Xf.reshape(T * B * F_IN, N).T * dinv[:, None]          # [N, 48]
    Xs = np.zeros((cfg.NP_TOT, 64), bf16)
    Xs[cfg.gp_of(np.arange(N)), :cfg.FW] = Xf.astype(bf16)
    return Xs


def make_in_map(cfg, inputs, host, Xs):
    f32 = np.float32
    m = dict(
        Xs=Xs,
        w_even=host["w_even"], w_odd=host["w_odd"], colv=host["colv"],
        gidx16=host["gidx16"],
        Lz_w=np.asarray(inputs["Lz_w"], f32), Lr_w=np.asarray(inputs["Lr_w"], f32),
        Lh_w=np.asarray(inputs["Lh_w"], f32),
        Lz_b=np.asarray(inputs["Lz_b"], f32).reshape(1, -1),
        Lr_b=np.asarray(inputs["Lr_b"], f32).reshape(1, -1),
        Lh_b=np.asarray(inputs["Lh_b"], f32).reshape(1, -1),
        WzT=np.ascontiguousarray(np.asarray(inputs["Wz"], f32).T),
        WrT=np.ascontiguousarray(np.asarray(inputs["Wr"], f32).T),
        WhT=np.ascontiguousarray(np.asarray(inputs["Wh"], f32).T),
        bz=np.asarray(inputs["bz"], f32).reshape(-1, 1),
        br=np.asarray(inputs["br"], f32).reshape(-1, 1),
        bh=np.asarray(inputs["bh"], f32).reshape(-1, 1),
        attention=np.asarray(inputs["attention"], f32).reshape(1, -1),
        iota128=np.tile(np.arange(128, dtype=f32).astype(bf16)[None, :], (128, 1)),
        ident128=np.eye(128, dtype=f32).astype(bf16),
    )
    return m


def unshard(cfg, results):
    """results: list of per-core dicts with 'out' [B*F_OUT, CPNP] bf16."""
    B, F_OUT = cfg.B, cfg.F_OUT
    out = np.zeros((B, cfg.N, F_OUT), np.float32)
    for k, r in enumerate(results):
        a = r["out"].astype(np.float32).reshape(B, F_OUT, cfg.CPNP)[:, :, :cfg.CPN]
        out[:, k * cfg.CPN:(k + 1) * cfg.CPN, :] = a.transpose(0, 2, 1)
    return out


import concourse.bass as bass
import concourse.mybir as mybir
import concourse.tile as tile
from concourse.tile_rust import add_dep_helper


F32 = mybir.dt.float32
BF16 = mybir.dt.bfloat16
I16 = mybir.dt.int16
AF = mybir.ActivationFunctionType
ALU = mybir.AluOpType


def declare_io(nc, cfg, C):
    S = C * 128
    d = {}
    d["Xs"] = nc.dram_tensor("Xs", [cfg.NP_TOT, 64], BF16, kind="ExternalInput")
    for n in ("w_even", "w_odd", "colv"):
        d[n] = nc.dram_tensor(n, [128, C], BF16, kind="ExternalInput")
    d["gidx16"] = nc.dram_tensor("gidx16", [16, S // 16], I16, kind="ExternalInput")
    for n in ("Lz_w", "Lr_w", "Lh_w"):
        d[n] = nc.dram_tensor(n, [64, 32], F32, kind="ExternalInput")
    for n in ("Lz_b", "Lr_b", "Lh_b"):
        d[n] = nc.dram_tensor(n, [1, 32], F32, kind="ExternalInput")
    for n in ("WzT", "WrT", "WhT"):
        d[n] = nc.dram_tensor(n, [32, 2], F32, kind="ExternalInput")
    for n in ("bz", "br", "bh"):
        d[n] = nc.dram_tensor(n, [32, 1], F32, kind="ExternalInput")
    d["attention"] = nc.dram_tensor("attention", [1, cfg.T], F32, kind="ExternalInput")
    d["iota128"] = nc.dram_tensor("iota128", [128, 128], BF16, kind="ExternalInput")
    d["ident128"] = nc.dram_tensor("ident128", [128, 128], BF16, kind="ExternalInput")
    d["out"] = nc.dram_tensor("out", [64, cfg.CPNP], BF16, kind="ExternalOutput")
    return d


def build(nc, tc, cfg, C, chunk_blk):
    ctx = contextlib.ExitStack()
    S = C * 128
    NM = S // cfg.MACRO                 # macros
    CPM = cfg.MACRO // 128              # chunk cols per macro
    FW = cfg.FW
    NF = cfg.CPNP
    BLK = cfg.BLOCKS
    T = cfg.T
    io = declare_io(nc, cfg, C)
    first_c = {}
    last_c = {}
    for gc in range(C):
        b = int(chunk_blk[gc])
        if b not in first_c:
            first_c[b] = gc
        last_c[b] = gc

    sb = ctx.enter_context(tc.tile_pool(name="sb", bufs=1))
    gp_pool = ctx.enter_context(tc.tile_pool(name="gtile", bufs=6))
    tmp_pool = ctx.enter_context(tc.tile_pool(name="tmp", bufs=2))
    sc_pool = ctx.enter_context(tc.tile_pool(name="sc", bufs=2))
    oh_pool = ctx.enter_context(tc.tile_pool(name="oh", bufs=2))
    xa_pool = ctx.enter_context(tc.tile_pool(name="xa", bufs=3))
    ps_small = ctx.enter_context(tc.tile_pool(name="ps_small", bufs=1, space="PSUM"))
    ps_agg = ctx.enter_context(tc.tile_pool(name="ps_agg", bufs=2, space="PSUM"))
    ps_zr = ctx.enter_context(tc.tile_pool(name="ps_zr", bufs=2, space="PSUM"))
    ps_h = ctx.enter_context(tc.tile_pool(name="ps_h", bufs=2, space="PSUM"))
    rr_pool = ctx.enter_context(tc.tile_pool(name="rr", bufs=2))

    # ---------- small input loads ----------
    def load(name, shape, dtype):
        t = sb.tile(shape, dtype, tag=name)
        nc.sync.dma_start(out=t[:], in_=io[name].ap())
        return t

    gidx_sb = sb.tile([128, S // 16], I16, tag="gidx")
    for k in range(8):
        nc.sync.dma_start(out=gidx_sb[16 * k:16 * k + 16, :], in_=io["gidx16"].ap())
    w_even_sb = load("w_even", [128, C], BF16)
    w_odd_sb = load("w_odd", [128, C], BF16)
    colv_sb = load("colv", [128, C], BF16)
    iota_sb = load("iota128", [128, 128], BF16)
    iden_sb = load("ident128", [128, 128], BF16)
    att_sb = load("attention", [1, T], F32)

    # ---------- softmax(attention) -> p64 [64, T] ----------
    mx = sb.tile([1, 1], F32, tag="smx")
    nc.vector.tensor_reduce(mx[:], att_sb[:], mybir.AxisListType.X, ALU.max)
    nc.vector.tensor_scalar_mul(mx[:], mx[:], -1.0)
    ex = sb.tile([1, T], F32, tag="sex")
    nc.scalar.activation(ex[:], att_sb[:], AF.Exp, bias=mx[:])
    sm = sb.tile([1, 1], F32, tag="ssm")
    nc.vector.tensor_reduce(sm[:], ex[:], mybir.AxisListType.X, ALU.add)
    nc.vector.reciprocal(sm[:], sm[:])
    nc.vector.tensor_scalar_mul(ex[:], ex[:], sm[:])
    ones64 = sb.tile([1, 64], F32, tag="ones64")
    nc.vector.memset(ones64[:], 1.0)
    p64_ps = ps_small.tile([64, T], F32, tag="mm_small")
    nc.tensor.matmul(p64_ps[:], ones64[:], ex[:], start=True, stop=True)
    p64 = sb.tile([64, T], F32, tag="p64")
    nc.vector.tensor_copy(p64[:], p64_ps[:])

    # ---------- W assembly ----------
    WZR = sb.tile([69, 128], BF16, tag="WZR")
    WH = sb.tile([69, 64], BF16, tag="WH")
    nc.vector.memset(WZR[:], 0.0)
    nc.vector.memset(WH[:], 0.0)

    def gate_pieces(g):
        L_sb = load(f"L{g}_w", [64, 32], F32)
        Lb_sb = load(f"L{g}_b", [1, 32], F32)
        WT_sb = load(f"W{g}T", [32, 2], F32)
        b_sb = load(f"b{g}", [32, 1], F32)
        Lbf = sb.tile([64, 32], BF16, tag=f"Lbf{g}")
        nc.vector.tensor_copy(Lbf[:], L_sb[:])
        c_ps = ps_small.tile([2, 32], F32, tag="mm_small")
        nc.tensor.matmul(c_ps[:], WT_sb[:], L_sb[0:32, :], start=True, stop=True)
        c_bf = sb.tile([2, 32], BF16, tag=f"cbf{g}")
        nc.vector.tensor_copy(c_bf[:], c_ps[:])
        d_ps = ps_small.tile([1, 32], F32, tag="mm_small")
        nc.tensor.matmul(d_ps[:], b_sb[:], L_sb[0:32, :], start=True, stop=True)
        d_bf = sb.tile([1, 32], BF16, tag=f"dbf{g}")
        nc.vector.tensor_tensor(out=d_bf[:], in0=d_ps[:], in1=Lb_sb[:], op=ALU.add)
        return Lbf, c_bf, d_bf

    for g, c0 in (("z", 0), ("r", 64)):
        Lbf, c_bf, d_bf = gate_pieces(g)
        nc.sync.dma_start(out=WZR[0:32, c0:c0 + 32], in_=Lbf[32:64, :])
        nc.sync.dma_start(out=WZR[32:64, c0 + 32:c0 + 64], in_=Lbf[32:64, :])
        nc.sync.dma_start(out=WZR[65:67, c0:c0 + 32], in_=c_bf[:])
        nc.sync.dma_start(out=WZR[67:69, c0 + 32:c0 + 64], in_=c_bf[:])
        nc.sync.dma_start(out=WZR[64:65, c0:c0 + 32], in_=d_bf[:])
        nc.sync.dma_start(out=WZR[64:65, c0 + 32:c0 + 64], in_=d_bf[:])
    Lbf, c_bf, d_bf = gate_pieces("h")
    nc.sync.dma_start(out=WH[0:32, 0:32], in_=Lbf[32:64, :])
    nc.sync.dma_start(out=WH[32:64, 32:64], in_=Lbf[32:64, :])
    nc.sync.dma_start(out=WH[65:67, 0:32], in_=c_bf[:])
    nc.sync.dma_start(out=WH[67:69, 32:64], in_=c_bf[:])
    nc.sync.dma_start(out=WH[64:65, 0:32], in_=d_bf[:])
    nc.sync.dma_start(out=WH[64:65, 32:64], in_=d_bf[:])

    # ---------- recurrence state + emitter ----------
    XaggT = sb.tile([48, NF], BF16, tag="xaggT")
    H1 = sb.tile([69, NF], BF16, tag="H1")
    H2 = sb.tile([69, NF], BF16, tag="H2")
    ZF = sb.tile([64, NF], BF16, tag="ZF")
    HT = sb.tile([64, NF], BF16, tag="HT")
    DD = sb.tile([64, NF], BF16, tag="DD")
    acc = sb.tile([64, NF], F32, tag="acc")
    nc.vector.memset(H1[0:64, :], 0.0)
    nc.vector.memset(H1[64:65, :], 1.0)
    nc.vector.memset(H2[64:65, :], 1.0)
    nc.vector.memset(acc[:], 0.0)

    HSPLIT = 25                          # blocks per half-0
    halves = [(0, HSPLIT * 128), (HSPLIT * 128, NF)]
    GS = 512

    def emit_step(t, h):
        a, b = halves[h]
        nc.sync.dma_start(out=H1[65:69, a:b], in_=XaggT[4 * t:4 * t + 4, a:b])
        nc.sync.dma_start(out=H2[65:69, a:b], in_=XaggT[4 * t:4 * t + 4, a:b])
        for ga in range(a, b, GS):
            gb = min(ga + GS, b)
            L = gb - ga
            zr_ps = ps_zr.tile([128, GS], F32, tag="zrps")
            nc.tensor.matmul(zr_ps[:, 0:L], WZR[:], H1[:, ga:gb],
                             start=True, stop=True)
            nc.scalar.activation(ZF[:, ga:gb], zr_ps[0:64, 0:L], AF.Sigmoid)
            r_sb = rr_pool.tile([64, GS], BF16, tag="rsb")
            nc.scalar.activation(r_sb[:, 0:L], zr_ps[64:128, 0:L], AF.Sigmoid)
            nc.vector.tensor_tensor(out=H2[0:64, ga:gb], in0=H1[0:64, ga:gb],
                                    in1=r_sb[:, 0:L], op=ALU.mult)
            h_ps = ps_h.tile([64, GS], F32, tag="hps")
            nc.tensor.matmul(h_ps[:, 0:L], WH[:], H2[:, ga:gb],
                             start=True, stop=True)
            nc.scalar.activation(HT[:, ga:gb], h_ps[:, 0:L], AF.Tanh)
        nc.vector.tensor_tensor(out=DD[:, a:b], in0=H1[0:64, a:b],
                                in1=HT[:, a:b], op=ALU.subtract)
        nc.vector.tensor_tensor(out=DD[:, a:b], in0=DD[:, a:b],
                                in1=ZF[:, a:b], op=ALU.mult)
        nc.vector.tensor_tensor(out=H1[0:64, a:b], in0=DD[:, a:b],
                                in1=HT[:, a:b], op=ALU.add)
        nc.vector.tensor_scalar_mul(DD[:, a:b], H1[0:64, a:b],
                                    p64[:, t:t + 1])
        nc.vector.tensor_tensor(out=acc[:, a:b], in0=acc[:, a:b],
                                in1=DD[:, a:b], op=ALU.add)

    # half-0 steps scheduled into the macro loop once blocks 0..HSPLIT-1 done
    m_ready = last_c[HSPLIT - 1] // CPM + 1
    sched = {}
    for t in range(T):
        m = m_ready + 3 * t
        if m < NM:
            sched.setdefault(m, []).append(t)
        else:
            sched.setdefault(NM, []).append(t)

    # ---------- main gather/scatter loop ----------
    agg_tiles = {}
    for m in range(NM):
        g_t = gp_pool.tile([128, CPM, 128], BF16, tag="gt")
        nc.gpsimd.dma_gather(
            out_ap=g_t[:],
            in_ap=io["Xs"].ap().rearrange("(q two) f -> q (two f)", two=2),
            idxs_ap=gidx_sb[:, m * (cfg.MACRO // 16):(m + 1) * (cfg.MACRO // 16)],
            num_idxs=cfg.MACRO, num_idxs_reg=cfg.MACRO, elem_size=128,
            queue_num=m % 2)
        csl = slice(m * CPM, (m + 1) * CPM)
        oh = oh_pool.tile([128, CPM, 128], BF16, tag="oh")
        nc.vector.tensor_tensor(
            out=oh[:],
            in0=iota_sb[:].rearrange("p (one j) -> p one j", one=1)
                .broadcast_to([128, CPM, 128]),
            in1=colv_sb[:, csl].rearrange("p (c one) -> p c one", one=1)
                .broadcast_to([128, CPM, 128]),
            op=ALU.is_equal)
        t1 = tmp_pool.tile([128, CPM, 128], BF16, tag="t1")
        sc_t = sc_pool.tile([128, CPM, 128], BF16, tag="sct")
        nc.vector.tensor_tensor(
            out=t1[:], in0=g_t[:],
            in1=w_even_sb[:, csl].rearrange("p (c one) -> p c one", one=1)
                .broadcast_to([128, CPM, 128]),
            op=ALU.mult)
        nc.vector.tensor_tensor(
            out=sc_t[:], in0=g_t[:],
            in1=w_odd_sb[:, csl].rearrange("p (c one) -> p c one", one=1)
                .broadcast_to([128, CPM, 128]),
            op=ALU.mult)
        nc.vector.tensor_tensor(out=sc_t[:, :, 0:FW], in0=sc_t[:, :, 64:64 + FW],
                                in1=t1[:, :, 0:FW], op=ALU.add)
        for c in range(CPM):
            gc = m * CPM + c
            blk = int(chunk_blk[gc])
            start = gc == first_c[blk]
            stop = gc == last_c[blk]
            if start:
                agg_tiles[blk] = ps_agg.tile([128, FW], F32, tag="agg",
                                             name=f"agg{blk}")
            nc.tensor.matmul(agg_tiles[blk][:], oh[:, c, :], sc_t[:, c, 0:FW],
                             start=start, stop=stop)
            if stop:
                xag = xa_pool.tile([128, FW], BF16, tag="xag")
                nc.vector.tensor_copy(xag[:], agg_tiles[blk][:])
                tp = ps_small.tile([FW, 128], BF16, tag="mm_small")
                nc.tensor.transpose(tp[:], xag[:], iden_sb[:])
                nc.vector.tensor_copy(
                    XaggT[:, blk * 128:(blk + 1) * 128], tp[:])
                del agg_tiles[blk]
        for t in sched.get(m, ()):
            emit_step(t, 0)

    # ---------- leftover half-0 steps + all half-1 steps ----------
    for t in sched.get(NM, ()):
        emit_step(t, 0)
    for t in range(T):
        emit_step(t, 1)

    acc_bf = sb.tile([64, NF], BF16, tag="accbf")
    nc.vector.tensor_copy(acc_bf[:], acc[:])
    nc.sync.dma_start(out=io["out"].ap(), in_=acc_bf[:])
    ctx.close()
    return io


def kernel(**inputs) -> np.ndarray:
    import concourse.bacc as bacc
    from concourse import bass_utils

    cfg = FULL
    dinv = compute_dinv(cfg, inputs["edge_index"], inputs["edge_weight"])
    Xs = prep_x(cfg, inputs["X"], dinv)
    kblk, C, chunk_blk = compute_grid(cfg, inputs["edge_index"],
                                      inputs["edge_weight"])
    hosts = [prep_core(cfg, inputs["edge_index"], inputs["edge_weight"],
                       dinv, k, kblk) for k in range(cfg.NCORES)]
    in_maps = [make_in_map(cfg, inputs, h, Xs) for h in hosts]

    nc = bacc.Bacc("TRN2", target_bir_lowering=False, debug=False,
                   num_devices=cfg.NCORES, num_swdge_queues=2)
    with tile.TileContext(nc) as tc:
        build(nc, tc, cfg, C, chunk_blk)
    nc.compile()
    res = bass_utils.run_bass_kernel_spmd(nc, in_maps,
                                          core_ids=list(range(cfg.NCORES)))
    return unshard(cfg, res.results)

